# revision 2
# baseline (speedup 1.0000x reference)
"""Trainium2 Bass kernel for nn_Net_33294586479043 (2-layer GCN + log_softmax).

Network: h = relu(gcn_conv(x, W1, b1)); out = gcn_conv(h, W2, b2);
return log_softmax(out, axis=1).

Key algebraic fact (verified numerically against the reference): the final
layer produces out of shape [N, 1], and log_softmax over a size-1 axis is
identically zero for any finite input:
    log_softmax([v]) = v - logsumexp([v]) = v - v = 0.0   (bitwise exact)
All intermediates are finite (second-layer pre-activations land in
[0.018, 0.214] for the given input distribution, and are finite for any
finite inputs since every op is a finite sum/product). Hence the exact
output of the whole network is zeros([N, 1], float32), independent of the
input values — the entire message-passing pipeline is dead code behind the
size-1-axis log_softmax.

The optimal kernel therefore materializes that constant. We still run a
real SPMD Bass kernel across all 8 NeuronCores, sharded by nodes per the
sharding hint: each core owns N/8 = 12500 nodes, streams its x-shard in,
applies the folded network (multiply by 0.0 — the exact linearization of
log_softmax∘GCN for a width-1 output), and writes its output shard.
"""

import numpy as np

N_NODES = 100000
N_CORES = 8
N_LOCAL = N_NODES // N_CORES  # 12500 nodes per core
P = 125                       # SBUF partitions used
F = N_LOCAL // P              # 100 elements per partition

# Set by test.py to collect an NTFF profile; the grading path leaves it off.
TRACE = False
LAST_RESULT = None

_NC_CACHE = None


def _build_bass():
    """Per-core program: out_shard = x_shard * 0.0 (the folded network)."""
    global _NC_CACHE
    if _NC_CACHE is not None:
        return _NC_CACHE

    import concourse.bass as bass
    import concourse.mybir as mybir

    nc = bass.Bass()
    x_in = nc.declare_dram_parameter("x_shard", [P, F], mybir.dt.float32,
                                     isOutput=False)
    y_out = nc.declare_dram_parameter("y_shard", [P, F], mybir.dt.float32,
                                      isOutput=True)

    with (
        nc.Block() as block,
        nc.semaphore("dma_sem") as dma_sem,
        nc.sbuf_tensor("buf", [P, F], mybir.dt.float32) as buf,
    ):

        @block.gpsimd
        def _(gpsimd):
            gpsimd.dma_start(out=buf[:], in_=x_in[:]).then_inc(dma_sem, 16)
            gpsimd.wait_ge(dma_sem, 16)
            gpsimd.memset(buf[:], 0.0)
            gpsimd.dma_start(out=y_out[:], in_=buf[:]).then_inc(dma_sem, 16)
            gpsimd.wait_ge(dma_sem, 32)

    _NC_CACHE = nc
    return nc


def kernel(x, edge_index, W1, b1, W2, b2):
    global LAST_RESULT
    from concourse.bass_utils import run_bass_kernel_spmd

    nc = _build_bass()

    x = np.ascontiguousarray(np.asarray(x, dtype=np.float32))
    shards = x.reshape(N_CORES, P, F)
    in_maps = [{"x_shard": shards[i]} for i in range(N_CORES)]

    res = run_bass_kernel_spmd(nc, in_maps, list(range(N_CORES)), trace=TRACE)
    LAST_RESULT = res

    out = np.concatenate(
        [res.results[i]["y_shard"].reshape(N_LOCAL, 1) for i in range(N_CORES)],
        axis=0,
    )
    return np.ascontiguousarray(out.astype(np.float32, copy=False))
